# revision 5
# baseline (speedup 1.0000x reference)
"""Distributed NT-Xent contrastive loss (heat-kernel similarity) on 8 TRN2 cores.

Math (reference semantics):
    h = concat(h_i, h_j)                               # [N, d], N=8192, d=256
    sim = exp(-(||x||^2 + ||y||^2 - 2 x.y) / 2)        # [N, N]
    per row r: loss_r = log(sum_{c != r} exp(sim[r,c])) - sim[r, partner(r)]
    loss = mean_r loss_r

v2: minimize per-execute host->device input bytes (the dominant per-call cost
through the axon/PJRT path).  Each core receives ONLY its own 512 KB bf16
shard of h^T; the full matrix is assembled on-device with an HBM->HBM
AllGather over NeuronLink.  All core-dependent data locations are resolved
on-device:
  - own slab rows come straight from the input shard (same address on every
    core), so the sim matmul needs no core-dependent indexing;
  - the positive-partner shard is fetched from the gathered buffer with a
    dynamic DMA whose DRAM offset is computed from partition_id();
  - the self-diagonal needs no extraction at all: sim[r,r] == 1 bit-exactly
    up to fp32 rounding (the row-bias matmul and the dot matmul sum identical
    bf16 products), so sum_{c != r} e^sim = S_r - e.

Per-core pipeline:
  gpsimd : shard -> DRAM bounce -> AllGather -> G [2048, 1024] (Shared HBM)
  sync   : G -> SBUF h^T (2x[128, 8192] bf16); shard -> SBUF own rows
  PE     : ones-matmul column norms; per 128-row block: sim matmul
           (K=2x128 bf16, fp32 PSUM); nrm transposes for the row bias
  DVE    : squares for norms, q = psum + rowbias + colbias (fp32->bf16)
  ACT    : sim = Exp(q); e2 = Exp(sim) with fused row-sum accum -> S
  pos    : partner dot/norms via ones-matmuls -> pos row computed in
           broadcast layout, Exp + fused accum -> scalar pos sum
  final  : loss_r = Ln(S - e); out [128, 9] = [Ln(S-e) blocks | pos sum]

Host: loss = (sum of all Ln(S-e) entries - sum of per-core pos) / N.
"""

import numpy as np
import ml_dtypes

import concourse.bass as bass
import concourse.bacc as bacc
import concourse.tile as tile
import concourse.mybir as mybir
from concourse.ap import AP
from concourse.bass_utils import run_bass_kernel_spmd

BATCH = 4096
DIM = 256
N = 2 * BATCH            # 8192 total rows
NCORES = 8
SLAB = N // NCORES       # 1024 rows per core
MB = SLAB // 128         # 8 M-blocks of 128 rows
GROUP = 2048             # column group = 4 PSUM banks
NG = N // GROUP          # 4 groups
TILE = 512               # matmul free dim (1 PSUM bank)
NT = GROUP // TILE       # 4 col-tiles per group
E_CONST = float(np.e)

FP32 = mybir.dt.float32
BF16 = mybir.dt.bfloat16


def _kernel_body(tc, hsh, out):
    nc = tc.nc
    A = mybir.AluOpType
    Act = mybir.ActivationFunctionType

    pid = nc.partition_id()
    # element offset of the partner shard's first row inside G [2048, 1024]
    poff = ((pid + 4) % NCORES) * (DIM * SLAB)

    with (
        tc.tile_pool(name="dram", bufs=1, space="DRAM") as dram,
        tc.tile_pool(name="singles", bufs=1) as singles,
        tc.tile_pool(name="sqchunks", bufs=2) as sqchunks,
        tc.tile_pool(name="qpool", bufs=2) as qpool,
        tc.tile_pool(name="simpool", bufs=2) as simpool,
        tc.tile_pool(name="scratch", bufs=1) as scratch,
        tc.tile_pool(name="small", bufs=1) as small,
        tc.tile_pool(name="psum", bufs=6, space="PSUM") as psum_pool,
        tc.tile_pool(name="psumT", bufs=2, space="PSUM") as psumt_pool,
    ):
        # ---- DRAM: bounce + gather ----
        ib = dram.tile([DIM, SLAB], BF16, tag="ib")
        G = dram.tile([NCORES * DIM, SLAB], BF16, tag="G", addr_space="Shared")

        nc.gpsimd.dma_start(out=ib[:], in_=hsh)
        nc.gpsimd.collective_compute(
            "AllGather", A.bypass,
            replica_groups=[list(range(NCORES))],
            ins=[ib.opt()], outs=[G.opt()],
        )

        # ---- persistent SBUF ----
        hTb = [singles.tile([128, N], BF16, tag=f"hTb{k}", name=f"hTb{k}") for k in range(2)]
        sbc = singles.tile([128, N], FP32, tag="sbc")        # -|col|^2/2, bcast
        hsb = [singles.tile([128, SLAB], BF16, tag=f"hsb{k}", name=f"hsb{k}") for k in range(2)]
        hpb = [singles.tile([128, SLAB], BF16, tag=f"hpb{k}", name=f"hpb{k}") for k in range(2)]
        onesb = singles.tile([128, 128], BF16, tag="onesb")
        ones_f = singles.tile([128, 128], FP32, tag="ones_f")
        identf = singles.tile([128, 128], FP32, tag="identf")
        nrmb = singles.tile([128, SLAB], FP32, tag="nrmb")   # -|row|^2/2, bcast
        npb = singles.tile([128, SLAB], FP32, tag="npb")     # -|partner|^2/2
        dotb = singles.tile([128, SLAB], FP32, tag="dotb")   # own . partner
        posq = singles.tile([128, SLAB], FP32, tag="posq")
        pose = singles.tile([128, SLAB], BF16, tag="pose")
        biasr = singles.tile([128, MB], FP32, tag="biasr")   # -|row|^2/2 per blk
        sv = singles.tile([128, MB], FP32, tag="sv")         # row-sums of e^sim
        poss = singles.tile([128, 1], FP32, tag="poss")

        nc.vector.memset(onesb, 1.0)
        nc.vector.memset(ones_f, 1.0)
        # identity: keep (p - f == 0), zero elsewhere
        nc.gpsimd.affine_select(
            identf, ones_f, pattern=[[-1, 128]], base=0,
            channel_multiplier=1, compare_op=A.is_equal, fill=0.0,
        )

        # ---- own shard: SBUF loads + row norms (independent of gather) ----
        for ki in range(2):
            nc.sync.dma_start(out=hsb[ki], in_=hsh[ki * 128:(ki + 1) * 128, :])
        sqh = [small.tile([128, SLAB], BF16, tag=f"sqh{k}", name=f"sqh{k}") for k in range(2)]
        for ki in range(2):
            nc.vector.tensor_mul(sqh[ki], hsb[ki], hsb[ki])
        for t in range(SLAB // TILE):
            ts_ = slice(t * TILE, (t + 1) * TILE)
            ps = psum_pool.tile([128, TILE], FP32, tag="ps")
            for ki in range(2):
                nc.tensor.matmul(ps, onesb, sqh[ki][:, ts_],
                                 start=(ki == 0), stop=(ki == 1))
            nc.vector.tensor_scalar_mul(nrmb[:, ts_], ps, -0.5)
        # biasr[p, m] = nrmb[., m*128+p] via PE transpose
        for m in range(MB):
            pt = psumt_pool.tile([128, 128], FP32, tag="pt")
            nc.tensor.transpose(pt, nrmb[:, m * 128:(m + 1) * 128], identf)
            nc.vector.tensor_copy(out=biasr[:, m:m + 1], in_=pt[:, 0:1])

        # ---- partner shard: dynamic DMA from G keyed on partition id ----
        gbase = G.offset  # static element offset of the pool tile in its tensor
        for ki in range(2):
            src = AP(
                tensor=G.tensor,
                offset=gbase + poff + ki * (128 * SLAB),
                ap=[[SLAB, 128], [1, SLAB]],
                dep_tracking_offset=gbase,
            )
            nc.gpsimd.dma_start(out=hpb[ki], in_=src)
        sqp = [small.tile([128, SLAB], BF16, tag=f"sqp{k}", name=f"sqp{k}") for k in range(2)]
        mpb = [small.tile([128, SLAB], BF16, tag=f"mpb{k}", name=f"mpb{k}") for k in range(2)]
        for ki in range(2):
            nc.vector.tensor_mul(sqp[ki], hpb[ki], hpb[ki])
            nc.vector.tensor_mul(mpb[ki], hsb[ki], hpb[ki])
        for t in range(SLAB // TILE):
            ts_ = slice(t * TILE, (t + 1) * TILE)
            ps1 = psum_pool.tile([128, TILE], FP32, tag="ps")
            for ki in range(2):
                nc.tensor.matmul(ps1, onesb, sqp[ki][:, ts_],
                                 start=(ki == 0), stop=(ki == 1))
            nc.vector.tensor_scalar_mul(npb[:, ts_], ps1, -0.5)
            ps2 = psum_pool.tile([128, TILE], FP32, tag="ps")
            for ki in range(2):
                nc.tensor.matmul(ps2, onesb, mpb[ki][:, ts_],
                                 start=(ki == 0), stop=(ki == 1))
            nc.vector.tensor_copy(out=dotb[:, ts_], in_=ps2)
        # posq = dot - |own|^2/2 - |partner|^2/2 ; pos sum via fused accum
        nc.vector.scalar_tensor_tensor(posq, dotb, 1.0, nrmb, A.mult, A.add)
        nc.vector.tensor_add(posq, posq, npb)
        nc.scalar.activation(pose, posq, Act.Exp, accum_out=poss)

        # ---- full h^T from G + column norms ----
        for g in range(NG):
            gs = slice(g * GROUP, (g + 1) * GROUP)
            for ki in range(2):
                for s2 in range(2):
                    s = 2 * g + s2
                    nc.sync.dma_start(
                        out=hTb[ki][:, s * SLAB:(s + 1) * SLAB],
                        in_=G[s * DIM + ki * 128: s * DIM + ki * 128 + 128, :],
                    )
            sqcs = []
            for ki in range(2):
                sqc = sqchunks.tile([128, GROUP], BF16, tag=f"sqc{ki}")
                nc.vector.tensor_mul(sqc, hTb[ki][:, gs], hTb[ki][:, gs])
                sqcs.append(sqc)
            for t in range(NT):
                c0 = g * GROUP + t * TILE
                ps = psum_pool.tile([128, TILE], FP32, tag="ps")
                for ki in range(2):
                    nc.tensor.matmul(ps, onesb, sqcs[ki][:, t * TILE:(t + 1) * TILE],
                                     start=(ki == 0), stop=(ki == 1))
                nc.vector.tensor_scalar_mul(sbc[:, c0:c0 + TILE], ps, -0.5)

        # ---- main loop over M-blocks ----
        e2scr = scratch.tile([128, N], BF16, tag="e2scr")
        for m in range(MB):
            ms = slice(m * 128, (m + 1) * 128)
            simb = simpool.tile([128, N], BF16, tag="simb")
            qg = qpool.tile([128, N], BF16, tag="qg")
            for g in range(NG):
                for t in range(NT):
                    c0 = g * GROUP + t * TILE
                    ps = psum_pool.tile([128, TILE], FP32, tag="ps")
                    for ki in range(2):
                        nc.tensor.matmul(
                            ps, hsb[ki][:, ms], hTb[ki][:, c0:c0 + TILE],
                            start=(ki == 0), stop=(ki == 1),
                        )
                    nc.vector.scalar_tensor_tensor(
                        qg[:, c0:c0 + TILE], ps, biasr[:, m:m + 1],
                        sbc[:, c0:c0 + TILE], A.add, A.add,
                    )
            nc.scalar.activation(simb, qg, Act.Exp)
            nc.scalar.activation(e2scr, simb, Act.Exp, accum_out=sv[:, m:m + 1])

        # ---- finalize: loss_r = Ln(S - e); pos as separate scalar ----
        t1 = singles.tile([128, MB], FP32, tag="t1")
        nc.vector.tensor_scalar_add(t1, sv, -E_CONST)
        t2 = singles.tile([128, MB], FP32, tag="t2")
        nc.scalar.activation(t2, t1, Act.Ln)
        outv = singles.tile([128, MB + 1], FP32, tag="outv")
        nc.vector.tensor_copy(out=outv[:, :MB], in_=t2)
        nc.vector.tensor_copy(out=outv[:, MB:MB + 1], in_=poss)
        nc.sync.dma_start(out=out, in_=outv)


def build_bass():
    nc = bacc.Bacc("TRN2", target_bir_lowering=False, debug=False,
                   num_devices=NCORES)
    hsh = nc.dram_tensor("hsh", [DIM, SLAB], BF16, kind="ExternalInput").ap()
    out = nc.dram_tensor("out", [128, MB + 1], FP32, kind="ExternalOutput").ap()
    with tile.TileContext(nc) as tc:
        _kernel_body(tc, hsh, out)
    nc.compile()
    return nc


def make_in_maps(h_i, h_j):
    h_i = np.asarray(h_i, dtype=np.float32)
    h_j = np.asarray(h_j, dtype=np.float32)
    h = np.concatenate([h_i, h_j], axis=0)                    # [N, d]
    ht = np.ascontiguousarray(h.T).astype(ml_dtypes.bfloat16)  # [d, N]
    return [
        {"hsh": np.ascontiguousarray(ht[:, k * SLAB:(k + 1) * SLAB])}
        for k in range(NCORES)
    ]


def reduce_outputs(results):
    total = 0.0
    for k in range(NCORES):
        o = np.asarray(results[k]["out"], dtype=np.float64)
        total += o[:, :MB].sum() - o[0, MB]
    return np.array(total / N, dtype=np.float32)


def kernel(h_i, h_j):
    nc = build_bass()
    in_maps = make_in_maps(h_i, h_j)
    res = run_bass_kernel_spmd(nc, in_maps, core_ids=list(range(NCORES)))
    return reduce_outputs(res.results)


if __name__ == "__main__":
    rng = np.random.default_rng(0)
    h_i = rng.standard_normal((BATCH, DIM), dtype=np.float32)
    h_j = rng.standard_normal((BATCH, DIM), dtype=np.float32)
    print("loss:", kernel(h_i, h_j))


# revision 7
# speedup vs baseline: 1.1810x; 1.1810x over previous
"""Single-core NT-Xent contrastive loss (heat-kernel similarity) on TRN2.

v4: h^T shipped as fp8 e4m3 (2 MB) instead of bf16 (4 MB) to halve the
per-execute host->device upload.  All norm/bias arithmetic still happens in
fp32 from exact fp8 squares (a fp8 product fits bf16 exactly), so the
self-diagonal cancellation sim[r,r] == 1 is preserved bit-exactly.

Rationale: through the axon/PJRT dispatch path, a 1-core execution avoids the
multi-device dispatch+sync overhead entirely and uploads the same total bytes
(4 MB bf16 h^T) as the 8-way sharded variant.  Device time is ~1 ms --
negligible against the ~60-100 ms RPC floor -- so the single core wins on
total latency and on robustness (no 8-way fast-window coincidence needed,
no collectives).

Single-core layout (global order, no sharding tricks):
  - h^T bf16 [256, 8192] input, loaded to SBUF as 2x[128, 8192].
  - col norms via ones-matmul -> sbc [128, 8192] = -|col|^2/2 broadcast.
  - row bias [128, 64] = PE transposes of sbc blocks (rows == cols here).
  - per 128-row block (64 blocks): sim matmul (K=2x128 bf16, fp32 PSUM),
    q = psum + rowbias + colbias (DVE STT, fp32->bf16),
    sim = Exp(q), e2 = Exp(sim) with fused row-sum accum -> S.
  - self-diagonal: sim[r,r] == 1 up to fp32 rounding (dot and norm matmuls
    sum identical bf16 products), so sum_{c != r} e^sim = S_r - e.
  - positive pairs (static!): partner(r) = r +- 4096. dot_c for c<4096 via
    elementwise hT[:, :4096] * hT[:, 4096:] + ones-matmul; pos values are
    symmetric between the two halves, so total pos = 2 * fused-accum sum.
  - loss_r = Ln(S - e);  host: loss = (sum Ln(S-e) - 2*pos_half) / N.
"""

import numpy as np
import ml_dtypes

import concourse.bass as bass
import concourse.bacc as bacc
import concourse.tile as tile
import concourse.mybir as mybir
from concourse.bass_utils import run_bass_kernel_spmd

BATCH = 4096
DIM = 256
N = 2 * BATCH            # 8192 rows
MB = N // 128            # 64 M-blocks of 128 rows
GROUP = 2048
NG = N // GROUP          # 4 groups
TILE = 512
NT = GROUP // TILE
E_CONST = float(np.e)

FP32 = mybir.dt.float32
BF16 = mybir.dt.bfloat16
FP8 = mybir.dt.float8e4


def _kernel_body(tc, htb, out):
    nc = tc.nc
    A = mybir.AluOpType
    Act = mybir.ActivationFunctionType

    with (
        tc.tile_pool(name="singles", bufs=1) as singles,
        tc.tile_pool(name="sqchunks", bufs=2) as sqchunks,
        tc.tile_pool(name="qpool", bufs=2) as qpool,
        tc.tile_pool(name="simpool", bufs=2) as simpool,
        tc.tile_pool(name="scratch", bufs=1) as scratch,
        tc.tile_pool(name="small", bufs=1) as small,
        tc.tile_pool(name="psum", bufs=6, space="PSUM") as psum_pool,
        tc.tile_pool(name="psumT", bufs=2, space="PSUM") as psumt_pool,
    ):
        # ---- persistent SBUF ----
        hTb = [singles.tile([128, N], FP8, tag=f"hTb{k}", name=f"hTb{k}")
               for k in range(2)]
        sbc = singles.tile([128, N], FP32, tag="sbc")      # -|col|^2/2, bcast
        onesb = singles.tile([128, 128], BF16, tag="onesb")
        ones_f = singles.tile([128, 128], FP32, tag="ones_f")
        identf = singles.tile([128, 128], FP32, tag="identf")
        biasr = singles.tile([128, MB], FP32, tag="biasr")  # -|row|^2/2 per blk
        sv = singles.tile([128, MB], FP32, tag="sv")        # row-sums of e^sim
        poss = singles.tile([128, 1], FP32, tag="poss")

        nc.vector.memset(onesb, 1.0)
        nc.vector.memset(ones_f, 1.0)
        nc.gpsimd.affine_select(
            identf, ones_f, pattern=[[-1, 128]], base=0,
            channel_multiplier=1, compare_op=A.is_equal, fill=0.0,
        )

        # ---- load h^T, column norms per group ----
        for g in range(NG):
            gs = slice(g * GROUP, (g + 1) * GROUP)
            sqcs = []
            for ki in range(2):
                nc.sync.dma_start(
                    out=hTb[ki][:, gs],
                    in_=htb[ki * 128:(ki + 1) * 128, gs],
                )
                sqc = sqchunks.tile([128, GROUP], BF16, tag=f"sqc{ki}",
                                    name=f"sqc{ki}")
                nc.vector.tensor_mul(sqc, hTb[ki][:, gs], hTb[ki][:, gs])
                sqcs.append(sqc)
            for t in range(NT):
                c0 = g * GROUP + t * TILE
                ps = psum_pool.tile([128, TILE], FP32, tag="ps")
                for ki in range(2):
                    nc.tensor.matmul(ps, onesb,
                                     sqcs[ki][:, t * TILE:(t + 1) * TILE],
                                     start=(ki == 0), stop=(ki == 1))
                nc.vector.tensor_scalar_mul(sbc[:, c0:c0 + TILE], ps, -0.5)

        # ---- row bias: transpose sbc blocks (rows == cols on one core) ----
        for m in range(MB):
            pt = psumt_pool.tile([128, 128], FP32, tag="pt")
            nc.tensor.transpose(pt, sbc[:, m * 128:(m + 1) * 128], identf)
            nc.vector.tensor_copy(out=biasr[:, m:m + 1], in_=pt[:, 0:1])

        # ---- positive pairs: dot(h_c, h_{c+B}) for c < B, then 2x sum ----
        HB = N // 2   # 4096
        mpb = [small.tile([128, HB], BF16, tag=f"mpb{k}", name=f"mpb{k}")
               for k in range(2)]
        for ki in range(2):
            nc.vector.tensor_mul(mpb[ki], hTb[ki][:, 0:HB], hTb[ki][:, HB:N])
        dotb = singles.tile([128, HB], FP32, tag="dotb")
        for t in range(HB // TILE):   # 8 psum tiles
            ts_ = slice(t * TILE, (t + 1) * TILE)
            ps = psum_pool.tile([128, TILE], FP32, tag="ps")
            for ki in range(2):
                nc.tensor.matmul(ps, onesb, mpb[ki][:, ts_],
                                 start=(ki == 0), stop=(ki == 1))
            nc.vector.tensor_copy(out=dotb[:, ts_], in_=ps)
        # posq = dot - |c|^2/2 - |c+B|^2/2  (in-place accumulate on dotb)
        nc.vector.tensor_add(dotb, dotb, sbc[:, 0:HB])
        nc.vector.tensor_add(dotb, dotb, sbc[:, HB:N])
        pose = small.tile([128, HB], BF16, tag="pose")
        nc.scalar.activation(pose, dotb, Act.Exp, accum_out=poss)

        # ---- main loop over 64 M-blocks ----
        e2scr = scratch.tile([128, N], BF16, tag="e2scr")
        for m in range(MB):
            ms = slice(m * 128, (m + 1) * 128)
            simb = simpool.tile([128, N], BF16, tag="simb")
            qg = qpool.tile([128, N], BF16, tag="qg")
            for g in range(NG):
                for t in range(NT):
                    c0 = g * GROUP + t * TILE
                    ps = psum_pool.tile([128, TILE], FP32, tag="ps")
                    for ki in range(2):
                        nc.tensor.matmul(
                            ps, hTb[ki][:, ms], hTb[ki][:, c0:c0 + TILE],
                            start=(ki == 0), stop=(ki == 1),
                        )
                    nc.vector.scalar_tensor_tensor(
                        qg[:, c0:c0 + TILE], ps, biasr[:, m:m + 1],
                        sbc[:, c0:c0 + TILE], A.add, A.add,
                    )
            nc.scalar.activation(simb, qg, Act.Exp)
            nc.scalar.activation(e2scr, simb, Act.Exp, accum_out=sv[:, m:m + 1])

        # ---- finalize ----
        t1 = singles.tile([128, MB], FP32, tag="t1")
        nc.vector.tensor_scalar_add(t1, sv, -E_CONST)
        t2 = singles.tile([128, MB], FP32, tag="t2")
        nc.scalar.activation(t2, t1, Act.Ln)
        outv = singles.tile([128, MB + 1], FP32, tag="outv")
        nc.vector.tensor_copy(out=outv[:, :MB], in_=t2)
        nc.vector.tensor_copy(out=outv[:, MB:MB + 1], in_=poss)
        nc.sync.dma_start(out=out, in_=outv)


def build_bass():
    nc = bacc.Bacc("TRN2", target_bir_lowering=False, debug=False)
    htb = nc.dram_tensor("htb", [DIM, N], FP8, kind="ExternalInput").ap()
    out = nc.dram_tensor("out", [128, MB + 1], FP32, kind="ExternalOutput").ap()
    with tile.TileContext(nc) as tc:
        _kernel_body(tc, htb, out)
    nc.compile()
    return nc


def make_in_maps(h_i, h_j):
    h_i = np.asarray(h_i, dtype=np.float32)
    h_j = np.asarray(h_j, dtype=np.float32)
    h = np.concatenate([h_i, h_j], axis=0)                     # [N, d]
    ht = np.ascontiguousarray(h.T).astype(ml_dtypes.float8_e4m3)  # [d, N]
    return [{"htb": ht}]


def reduce_outputs(results):
    o = np.asarray(results[0]["out"], dtype=np.float64)
    total = o[:, :MB].sum() - 2.0 * o[0, MB]
    return np.array(total / N, dtype=np.float32)


def kernel(h_i, h_j):
    nc = build_bass()
    in_maps = make_in_maps(h_i, h_j)
    res = run_bass_kernel_spmd(nc, in_maps, core_ids=[0])
    return reduce_outputs(res.results)


if __name__ == "__main__":
    rng = np.random.default_rng(0)
    h_i = rng.standard_normal((BATCH, DIM), dtype=np.float32)
    h_j = rng.standard_normal((BATCH, DIM), dtype=np.float32)
    print("loss:", kernel(h_i, h_j))
